# revision 21
# baseline (speedup 1.0000x reference)
"""Cross-attention kernel for Trainium2 (8 NeuronCores, data-parallel over batch).

Problem (hardcoded): B=8, Sq=4096, Sk=77, E=1024, C=768, H=16 heads, D=64.

    q = x @ wq + bq; k = y @ wk + bk; v = y @ wv + bv
    out = softmax(q k^T / sqrt(D)) v @ wo + bo

Sharding: batch element b -> core b. No collectives.

Per-core device pipeline (all matmuls contract over the SBUF partition dim):
  - Activations are kept feature-major ("transposed"): xT[E, Sq] is prepared
    host-side, so QT = wq^T-free matmul chain produces qT[E, Sq] directly,
    per-head slices qT[h*64:(h+1)*64, :] feed scores without any on-chip
    transpose.
  - kT[E, Sk] via lhsT=wk_aug tiles; V[Sk, E] row-major via lhsT=yT_aug tiles.
    Biases for k/v are folded in by augmenting y with a ones-row (host side).
  - scores^T[Sk, q] = matmul(lhsT=kT head slice [64, 77], rhs=qT head slice).
    The 1/sqrt(D) scale is folded into wq/bq host-side.
  - softmax without max-subtraction (scores are O(5), exp is safe in fp32):
    exp on ScalarE; per-head denominators via one-hot [77, 16] matmuls
    accumulated into one PSUM tile; reciprocal on VectorE; broadcast across
    partitions via SBUF->SBUF DMA (stride-0 partition source).
  - o^T = matmul(lhsT=V head slice [77, 64], rhs=exps), normalized during
    PSUM eviction (DVE multiply by the broadcast reciprocal), written into
    oT[E, q] with even/odd heads at partition offsets 0/64 of paired tiles.
  - out[q, E] row-major = matmul(lhsT=oT tiles [128, 128], rhs=wo tiles),
    bias bo added during eviction from a partition-broadcast bias tile.

All matmul operands are typed float32r (fp32 with 11 mantissa bits): 1
cycle/row on the PE at N=512 (4x the plain-fp32 rate). Operands coming from
DRAM are pre-rounded host-side (round-to-nearest-even to the 20-bit format);
on-chip producers round by writing float32r-typed outputs.
"""

import os
from contextlib import ExitStack

import numpy as np

import concourse.bass as bass
import concourse.tile as tile
from concourse import bacc, mybir
from concourse.bass_utils import run_bass_kernel_spmd

N_CORES = 8
SQ = 4096
SK = 77
SKP = 80  # SK padded: fp32r matmul dst free-size must be even
E = 1024
C = 768
CA = 896  # C padded to 7*128, with the ones-row at index C
H = 16
D = 64
CHUNK = 512
NCHUNK = SQ // CHUNK  # 8
ET = E // 128  # 8 e-tiles
CT = CA // 128  # 7 c-tiles
F32 = mybir.dt.float32
F32R = mybir.dt.float32r

_PROGRAM = None


def _round_f32r(a: np.ndarray) -> np.ndarray:
    """Round fp32 to the fp32r format (11 mantissa bits, RNE)."""
    u = np.ascontiguousarray(a, dtype=np.float32).view(np.uint32).copy()
    u += np.uint32(0x7FF) + ((u >> np.uint32(12)) & np.uint32(1))
    u &= np.uint32(0xFFFFF000)
    return u.view(np.float32)


def _build_program():
    nc = bacc.Bacc(
        "TRN2", target_bir_lowering=False, debug=False, num_devices=N_CORES
    )
    xT_d = nc.dram_tensor("xT", [E, SQ], F32R, kind="ExternalInput").ap()
    yT_d = nc.dram_tensor("yT", [CA, SKP], F32R, kind="ExternalInput").ap()
    wq_d = nc.dram_tensor("wq", [E, E], F32R, kind="ExternalInput").ap()
    bq_d = nc.dram_tensor("bq", [E], F32, kind="ExternalInput").ap()
    wk_d = nc.dram_tensor("wk", [CA, E], F32R, kind="ExternalInput").ap()
    wv_d = nc.dram_tensor("wv", [CA, H * 64], F32R, kind="ExternalInput").ap()
    wo_d = nc.dram_tensor("wo", [E, E], F32R, kind="ExternalInput").ap()
    bo_d = nc.dram_tensor("bo", [E], F32, kind="ExternalInput").ap()
    oh_d = nc.dram_tensor("oh", [SK, H * H], F32R, kind="ExternalInput").ap()
    sel_d = nc.dram_tensor("sel", [H, ET * 128], F32R, kind="ExternalInput").ap()
    out_d = nc.dram_tensor("out", [SQ, E], F32, kind="ExternalOutput").ap()

    with tile.TileContext(nc) as tc, ExitStack() as ctx:
        consts = ctx.enter_context(tc.tile_pool(name="consts", bufs=1))
        wq_sb = consts.tile([128, ET, E], F32R)
        wo_sb = consts.tile([128, ET, E], F32R)
        kT_sb = consts.tile([128, ET, SKP], F32R)
        v_sb = consts.tile([SK, H * 64], F32R)
        oh_sb = consts.tile([SK, H * H], F32R)
        sel_sb = consts.tile([H, ET * 128], F32R)
        bq_sb = consts.tile([128, ET], F32)
        bo_sb = consts.tile([128, E], F32)

        nc.sync.dma_start(wq_sb[:], wq_d.rearrange("(t p) n -> p t n", p=128))
        nc.sync.dma_start(wo_sb[:], wo_d.rearrange("(t p) n -> p t n", p=128))
        nc.sync.dma_start(oh_sb[:], oh_d)
        nc.sync.dma_start(sel_sb[:], sel_d)
        nc.sync.dma_start(bq_sb[:], bq_d.rearrange("(t p) -> p t", p=128))
        nc.sync.dma_start(bo_sb[:], bo_d.partition_broadcast(128))

        # Phase 0: kT[E, Sk] and V[Sk, E] (k/v biases folded via y ones-row).
        with tc.tile_pool(name="ph0", bufs=1) as ph0, tc.tile_pool(
            name="ph0ps", bufs=2, space="PSUM"
        ) as ph0ps:
            yT_sb = ph0.tile([128, CT, SKP], F32R)
            wk_sb = ph0.tile([128, CT, E], F32R)
            wv_sb = ph0.tile([128, CT, H * 64], F32R)
            nc.sync.dma_start(yT_sb[:], yT_d.rearrange("(t p) n -> p t n", p=128))
            nc.sync.dma_start(wk_sb[:], wk_d.rearrange("(t p) n -> p t n", p=128))
            nc.sync.dma_start(wv_sb[:], wv_d.rearrange("(t p) n -> p t n", p=128))
            for et in range(ET):
                ps = ph0ps.tile([128, SKP], F32, tag="psk")
                for t in range(CT):
                    nc.tensor.matmul(
                        ps[:],
                        wk_sb[:, t, et * 128 : (et + 1) * 128],
                        yT_sb[:, t, :],
                        start=(t == 0),
                        stop=(t == CT - 1),
                    )
                nc.vector.tensor_copy(kT_sb[:, et, :], ps[:])
            for n0 in range(0, H * 64, CHUNK):
                ps = ph0ps.tile([SK, CHUNK], F32, tag="psv")
                for t in range(CT):
                    nc.tensor.matmul(
                        ps[:],
                        yT_sb[:, t, 0:SK],
                        wv_sb[:, t, n0 : n0 + CHUNK],
                        start=(t == 0),
                        stop=(t == CT - 1),
                    )
                nc.vector.tensor_copy(v_sb[:, n0 : n0 + CHUNK], ps[:])

        # Main loop over row chunks. Emission order per chunk:
        #   QT(c) -> final(c-1) -> attention(c)
        # keeps the PE busy with QT matmuls while chunk c-1's normalization
        # (DVE/DMA) completes.
        xT_pool = ctx.enter_context(tc.tile_pool(name="xT", bufs=2))
        qT_pool = ctx.enter_context(tc.tile_pool(name="qT", bufs=2))
        oT_pool = ctx.enter_context(tc.tile_pool(name="oT", bufs=2))
        exps_pool = ctx.enter_context(tc.tile_pool(name="exps", bufs=4))
        tmpb_pool = ctx.enter_context(tc.tile_pool(name="tmpb", bufs=3))
        recip_pool = ctx.enter_context(tc.tile_pool(name="recip", bufs=2))
        outs_pool = ctx.enter_context(tc.tile_pool(name="outs", bufs=2))
        ps_q = ctx.enter_context(tc.tile_pool(name="ps_q", bufs=2, space="PSUM"))
        ps_s = ctx.enter_context(tc.tile_pool(name="ps_s", bufs=2, space="PSUM"))
        ps_den = ctx.enter_context(tc.tile_pool(name="ps_den", bufs=1, space="PSUM"))
        ps_av = ctx.enter_context(tc.tile_pool(name="ps_av", bufs=2, space="PSUM"))
        ps_f = ctx.enter_context(tc.tile_pool(name="ps_f", bufs=1, space="PSUM"))

        def emit_final_group(c, oT_sb, i):
            qt, n0 = i // 2, (i % 2) * CHUNK
            ps = ps_f.tile([128, CHUNK], F32, tag="psf")
            for t in range(ET):
                nc.tensor.matmul(
                    ps[:],
                    oT_sb[:, t, qt * 128 : (qt + 1) * 128],
                    wo_sb[:, t, n0 : n0 + CHUNK],
                    start=(t == 0),
                    stop=(t == ET - 1),
                )
            o_sb = outs_pool.tile([128, CHUNK], F32, tag="osb")
            nc.vector.tensor_tensor(
                o_sb[:], ps[:], bo_sb[:, n0 : n0 + CHUNK], mybir.AluOpType.add
            )
            r0 = c * CHUNK + qt * 128
            nc.sync.dma_start(out_d[r0 : r0 + 128, n0 : n0 + CHUNK], o_sb[:])

        prev = None
        for c in range(NCHUNK):
            cs = slice(c * CHUNK, (c + 1) * CHUNK)
            xT_sb = xT_pool.tile([128, ET, CHUNK], F32R, tag="xT")
            nc.sync.dma_start(
                xT_sb[:], xT_d.rearrange("(t p) n -> p t n", p=128)[:, :, cs]
            )
            qT_sb = qT_pool.tile([128, ET, CHUNK], F32R, tag="qT")
            for et in range(ET):
                ps = ps_q.tile([128, CHUNK], F32, tag="psq")
                for t in range(ET):
                    nc.tensor.matmul(
                        ps[:],
                        wq_sb[:, t, et * 128 : (et + 1) * 128],
                        xT_sb[:, t, :],
                        start=(t == 0),
                        stop=(t == ET - 1),
                    )
                nc.vector.tensor_scalar(
                    qT_sb[:, et, :],
                    ps[:],
                    bq_sb[:, et : et + 1],
                    None,
                    mybir.AluOpType.add,
                )

            # Attention for chunk c, interleaved with chunk c-1's output
            # projection: the final-matmul groups have no dependency on this
            # chunk's exps, so they keep the PE busy (and the HAM clock-gate
            # warm) while the ScalarE exp of each head pair is in flight.
            #
            # fp32r matmuls must write PSUM at base partition 0, so each head
            # gets its own [64, CHUNK] attnV tile; the even head of a pair is
            # evicted to oT[0:64] by the DVE, the odd head by a
            # partition-shifting SBUF<-SBUF DMA to oT[64:128]. The softmax
            # division happens afterwards, in place on oT.
            pden = ps_den.tile([H, CHUNK], F32, tag="psden")
            oT_sb = oT_pool.tile([128, ET, CHUNK], F32R, tag="oT")
            for et in range(ET):
                hA, hB = 2 * et, 2 * et + 1
                psa = ps_s.tile([SK, CHUNK], F32, tag="pss")
                psb = ps_s.tile([SK, CHUNK], F32, tag="pss")
                # Adjacent score matmuls target PE row groups 0/64 and can
                # overlap in the array.
                nc.tensor.matmul(
                    psa[:], kT_sb[0:64, et, 0:SK], qT_sb[0:64, et, :],
                    start=True, stop=True,
                )
                nc.tensor.matmul(
                    psb[:], kT_sb[64:128, et, 0:SK], qT_sb[64:128, et, :],
                    start=True, stop=True,
                )
                exa = exps_pool.tile([SK, CHUNK], F32R, tag="exps")
                exb = exps_pool.tile([SK, CHUNK], F32R, tag="exps")
                nc.scalar.activation(exa[:], psa[:], mybir.ActivationFunctionType.Exp)
                nc.scalar.activation(exb[:], psb[:], mybir.ActivationFunctionType.Exp)
                for h, ex in ((hA, exa), (hB, exb)):
                    nc.tensor.matmul(
                        pden[:],
                        oh_sb[:, h * H : (h + 1) * H],
                        ex[:],
                        start=(h == 0),
                        stop=(h == H - 1),
                    )
                    pav = ps_av.tile([64, CHUNK], F32, tag="psav")
                    nc.tensor.matmul(
                        pav[:],
                        v_sb[:, h * 64 : (h + 1) * 64],
                        ex[:],
                        start=True,
                        stop=True,
                    )
                    if h == hA:
                        nc.vector.tensor_copy(oT_sb[0:64, et, :], pav[:])
                    else:
                        tmpb = tmpb_pool.tile([64, CHUNK], F32R, tag="tmpb")
                        nc.vector.tensor_copy(tmpb[:], pav[:])
                        nc.sync.dma_start(oT_sb[64:128, et, :], tmpb[:])
                if prev is not None:
                    emit_final_group(prev[0], prev[1], et)
            recip = recip_pool.tile([H, CHUNK], F32R, tag="recip")
            with nc.allow_low_precision(reason="fp32r feeds select-matmul"):
                nc.vector.reciprocal(recip[:], pden[:])
            # Broadcast recip rows (2*et, 2*et+1) across the pair's 128
            # partitions with a one-hot select matmul, then divide in place.
            for et in range(ET):
                rb = ps_s.tile([128, CHUNK], F32, tag="pss")
                nc.tensor.matmul(
                    rb[:],
                    sel_sb[:, et * 128 : (et + 1) * 128],
                    recip[:],
                    start=True,
                    stop=True,
                )
                nc.vector.tensor_tensor(
                    oT_sb[0:64, et, :],
                    oT_sb[0:64, et, :],
                    rb[0:64, :],
                    mybir.AluOpType.mult,
                )
                nc.vector.tensor_tensor(
                    oT_sb[64:128, et, :],
                    oT_sb[64:128, et, :],
                    rb[64:128, :],
                    mybir.AluOpType.mult,
                )
            prev = (c, oT_sb)
        for i in range(8):
            emit_final_group(prev[0], prev[1], i)

    nc.compile()
    return nc


def _get_program():
    global _PROGRAM
    if _PROGRAM is None:
        _PROGRAM = _build_program()
    return _PROGRAM


def kernel(x, y, wq, bq, wk, bk, wv, bv, wo, bo):
    x = np.asarray(x, dtype=np.float32)
    y = np.asarray(y, dtype=np.float32)
    wq = np.asarray(wq, dtype=np.float32)
    bq = np.asarray(bq, dtype=np.float32)
    wk = np.asarray(wk, dtype=np.float32)
    bk = np.asarray(bk, dtype=np.float32)
    wv = np.asarray(wv, dtype=np.float32)
    bv = np.asarray(bv, dtype=np.float32)
    wo = np.asarray(wo, dtype=np.float32)
    bo = np.asarray(bo, dtype=np.float32)

    scale = np.float32(1.0 / np.sqrt(D))
    wq_s = _round_f32r(wq * scale)
    bq_s = (bq * scale).astype(np.float32)

    wk_aug = np.zeros((CA, E), dtype=np.float32)
    wk_aug[:C] = wk
    wk_aug[C] = bk
    wk_aug = _round_f32r(wk_aug)

    wv_aug = np.zeros((CA, H * 64), dtype=np.float32)
    wv_aug[:C] = wv
    wv_aug[C] = bv
    wv_aug = _round_f32r(wv_aug)

    wo_r = _round_f32r(wo)

    onehot = np.zeros((SK, H, H), dtype=np.float32)
    for h in range(H):
        onehot[:, h, h] = 1.0
    onehot = onehot.reshape(SK, H * H)

    sel = np.zeros((H, ET, 128), dtype=np.float32)
    for et in range(ET):
        sel[2 * et, et, 0:64] = 1.0
        sel[2 * et + 1, et, 64:128] = 1.0
    sel = sel.reshape(H, ET * 128)

    nc = _get_program()
    in_maps = []
    for b in range(N_CORES):
        xT = _round_f32r(x[b].T)
        yT = np.zeros((CA, SKP), dtype=np.float32)
        yT[:C, :SK] = y[b].T
        yT[C, :SK] = 1.0
        yT = _round_f32r(yT)
        in_maps.append(
            {
                "xT": xT,
                "yT": yT,
                "wq": wq_s,
                "bq": bq_s,
                "wk": wk_aug,
                "wv": wv_aug,
                "wo": wo_r,
                "bo": bo,
                "oh": onehot,
                "sel": sel,
            }
        )

    trace = bool(int(os.environ.get("KERNEL_TRACE", "0")))
    kwargs = {}
    if trace:
        kwargs = {"trace": True, "tmpdir": os.environ.get("KERNEL_TRACE_DIR")}
    res = run_bass_kernel_spmd(nc, in_maps, list(range(N_CORES)), **kwargs)
    if trace:
        kernel.last_exec_time_ns = res.exec_time_ns
        kernel.last_results = res
    out = np.stack([res.results[b]["out"] for b in range(N_CORES)])
    return np.ascontiguousarray(out)


# revision 24
# speedup vs baseline: 1.0278x; 1.0278x over previous
"""Cross-attention kernel for Trainium2 (8 NeuronCores, data-parallel over batch).

Problem (hardcoded): B=8, Sq=4096, Sk=77, E=1024, C=768, H=16 heads, D=64.

    q = x @ wq + bq; k = y @ wk + bk; v = y @ wv + bv
    out = softmax(q k^T / sqrt(D)) v @ wo + bo

Sharding: batch element b -> core b. No collectives.

Per-core device pipeline (all matmuls contract over the SBUF partition dim):
  - Activations are kept feature-major ("transposed"): xT[E, Sq] is prepared
    host-side, so QT = wq^T-free matmul chain produces qT[E, Sq] directly,
    per-head slices qT[h*64:(h+1)*64, :] feed scores without any on-chip
    transpose.
  - kT[E, Sk] via lhsT=wk_aug tiles; V[Sk, E] row-major via lhsT=yT_aug tiles.
    Biases for k/v are folded in by augmenting y with a ones-row (host side).
  - scores^T[Sk, q] = matmul(lhsT=kT head slice [64, 77], rhs=qT head slice).
    The 1/sqrt(D) scale is folded into wq/bq host-side.
  - softmax without max-subtraction (scores are O(5), exp is safe in fp32):
    exp on ScalarE; per-head denominators via one-hot [77, 16] matmuls
    accumulated into one PSUM tile; reciprocal on VectorE; broadcast across
    partitions via SBUF->SBUF DMA (stride-0 partition source).
  - o^T = matmul(lhsT=V head slice [77, 64], rhs=exps), normalized during
    PSUM eviction (DVE multiply by the broadcast reciprocal), written into
    oT[E, q] with even/odd heads at partition offsets 0/64 of paired tiles.
  - out[q, E] row-major = matmul(lhsT=oT tiles [128, 128], rhs=wo tiles),
    bias bo added during eviction from a partition-broadcast bias tile.

All matmul operands are typed float32r (fp32 with 11 mantissa bits): 1
cycle/row on the PE at N=512 (4x the plain-fp32 rate). Operands coming from
DRAM are pre-rounded host-side (round-to-nearest-even to the 20-bit format);
on-chip producers round by writing float32r-typed outputs.
"""

import os
from contextlib import ExitStack

import numpy as np

import concourse.bass as bass
import concourse.tile as tile
from concourse import bacc, mybir
from concourse.bass_utils import run_bass_kernel_spmd

N_CORES = 8
SQ = 4096
SK = 77
SKP = 80  # SK padded: fp32r matmul dst free-size must be even
E = 1024
C = 768
CA = 896  # C padded to 7*128, with the ones-row at index C
H = 16
D = 64
CHUNK = 512
NCHUNK = SQ // CHUNK  # 8
ET = E // 128  # 8 e-tiles
CT = CA // 128  # 7 c-tiles
F32 = mybir.dt.float32
F32R = mybir.dt.float32r

_PROGRAM = None


def _round_f32r(a: np.ndarray) -> np.ndarray:
    """Round fp32 to the fp32r format (11 mantissa bits, RNE)."""
    u = np.ascontiguousarray(a, dtype=np.float32).view(np.uint32).copy()
    u += np.uint32(0x7FF) + ((u >> np.uint32(12)) & np.uint32(1))
    u &= np.uint32(0xFFFFF000)
    return u.view(np.float32)


def _build_program():
    nc = bacc.Bacc(
        "TRN2", target_bir_lowering=False, debug=False, num_devices=N_CORES
    )
    xT_d = nc.dram_tensor("xT", [E, SQ], F32R, kind="ExternalInput").ap()
    yT_d = nc.dram_tensor("yT", [CA, SKP], F32R, kind="ExternalInput").ap()
    wq_d = nc.dram_tensor("wq", [E, E], F32R, kind="ExternalInput").ap()
    bq_d = nc.dram_tensor("bq", [E], F32, kind="ExternalInput").ap()
    wk_d = nc.dram_tensor("wk", [CA, E], F32R, kind="ExternalInput").ap()
    wv_d = nc.dram_tensor("wv", [CA, H * 64], F32R, kind="ExternalInput").ap()
    wo_d = nc.dram_tensor("wo", [E, E], F32R, kind="ExternalInput").ap()
    bo_d = nc.dram_tensor("bo", [E], F32, kind="ExternalInput").ap()
    oh_d = nc.dram_tensor("oh", [SK, H * H], F32R, kind="ExternalInput").ap()
    sel_d = nc.dram_tensor("sel", [H, ET * 128], F32R, kind="ExternalInput").ap()
    out_d = nc.dram_tensor("out", [SQ, E], F32, kind="ExternalOutput").ap()

    with tile.TileContext(nc) as tc, ExitStack() as ctx:
        consts = ctx.enter_context(tc.tile_pool(name="consts", bufs=1))
        wq_sb = consts.tile([128, ET, E], F32R)
        wo_sb = consts.tile([128, ET, E], F32R)
        kT_sb = consts.tile([128, ET, SKP], F32R)
        v_sb = consts.tile([SK, H * 64], F32R)
        oh_sb = consts.tile([SK, H * H], F32R)
        sel_sb = consts.tile([H, ET * 128], F32R)
        bq_sb = consts.tile([128, ET], F32)
        bo_sb = consts.tile([128, E], F32)

        nc.sync.dma_start(oh_sb[:], oh_d)
        nc.sync.dma_start(sel_sb[:], sel_d)
        nc.sync.dma_start(bq_sb[:], bq_d.rearrange("(t p) -> p t", p=128))
        nc.sync.dma_start(bo_sb[:], bo_d.partition_broadcast(128))

        # Phase 0: kT[E, Sk] and V[Sk, E] (k/v biases folded via y ones-row).
        with tc.tile_pool(name="ph0", bufs=1) as ph0, tc.tile_pool(
            name="ph0ps", bufs=2, space="PSUM"
        ) as ph0ps:
            yT_sb = ph0.tile([128, CT, SKP], F32R)
            wk_sb = ph0.tile([128, CT, E], F32R)
            wv_sb = ph0.tile([128, CT, H * 64], F32R)
            nc.sync.dma_start(yT_sb[:], yT_d.rearrange("(t p) n -> p t n", p=128))
            nc.sync.dma_start(wk_sb[:], wk_d.rearrange("(t p) n -> p t n", p=128))
            nc.sync.dma_start(wv_sb[:], wv_d.rearrange("(t p) n -> p t n", p=128))
            nc.sync.dma_start(wq_sb[:], wq_d.rearrange("(t p) n -> p t n", p=128))
            nc.sync.dma_start(wo_sb[:], wo_d.rearrange("(t p) n -> p t n", p=128))
            for et in range(ET):
                ps = ph0ps.tile([128, SKP], F32, tag="psk")
                for t in range(CT):
                    nc.tensor.matmul(
                        ps[:],
                        wk_sb[:, t, et * 128 : (et + 1) * 128],
                        yT_sb[:, t, :],
                        start=(t == 0),
                        stop=(t == CT - 1),
                    )
                nc.vector.tensor_copy(kT_sb[:, et, :], ps[:])
            for n0 in range(0, H * 64, CHUNK):
                ps = ph0ps.tile([SK, CHUNK], F32, tag="psv")
                for t in range(CT):
                    nc.tensor.matmul(
                        ps[:],
                        yT_sb[:, t, 0:SK],
                        wv_sb[:, t, n0 : n0 + CHUNK],
                        start=(t == 0),
                        stop=(t == CT - 1),
                    )
                nc.vector.tensor_copy(v_sb[:, n0 : n0 + CHUNK], ps[:])

        # Main loop over row chunks. Emission order per chunk:
        #   QT(c) -> final(c-1) -> attention(c)
        # keeps the PE busy with QT matmuls while chunk c-1's normalization
        # (DVE/DMA) completes.
        xT_pool = ctx.enter_context(tc.tile_pool(name="xT", bufs=2))
        qT_pool = ctx.enter_context(tc.tile_pool(name="qT", bufs=2))
        oT_pool = ctx.enter_context(tc.tile_pool(name="oT", bufs=2))
        exps_pool = ctx.enter_context(tc.tile_pool(name="exps", bufs=4))
        tmpb_pool = ctx.enter_context(tc.tile_pool(name="tmpb", bufs=3))
        recip_pool = ctx.enter_context(tc.tile_pool(name="recip", bufs=2))
        outs_pool = ctx.enter_context(tc.tile_pool(name="outs", bufs=2))
        ps_q = ctx.enter_context(tc.tile_pool(name="ps_q", bufs=2, space="PSUM"))
        ps_s = ctx.enter_context(tc.tile_pool(name="ps_s", bufs=2, space="PSUM"))
        ps_den = ctx.enter_context(tc.tile_pool(name="ps_den", bufs=1, space="PSUM"))
        ps_av = ctx.enter_context(tc.tile_pool(name="ps_av", bufs=2, space="PSUM"))
        ps_f = ctx.enter_context(tc.tile_pool(name="ps_f", bufs=1, space="PSUM"))

        def emit_final_group(c, oT_sb, i):
            qt, n0 = i // 2, (i % 2) * CHUNK
            ps = ps_f.tile([128, CHUNK], F32, tag="psf")
            for t in range(ET):
                nc.tensor.matmul(
                    ps[:],
                    oT_sb[:, t, qt * 128 : (qt + 1) * 128],
                    wo_sb[:, t, n0 : n0 + CHUNK],
                    start=(t == 0),
                    stop=(t == ET - 1),
                )
            o_sb = outs_pool.tile([128, CHUNK], F32, tag="osb")
            nc.vector.tensor_tensor(
                o_sb[:], ps[:], bo_sb[:, n0 : n0 + CHUNK], mybir.AluOpType.add
            )
            r0 = c * CHUNK + qt * 128
            nc.sync.dma_start(out_d[r0 : r0 + 128, n0 : n0 + CHUNK], o_sb[:])

        xT_r = xT_d.rearrange("(t p) n -> p t n", p=128)

        def load_xT(c):
            cs = slice(c * CHUNK, (c + 1) * CHUNK)
            xT_sb = xT_pool.tile([128, ET, CHUNK], F32R, tag="xT")
            nc.sync.dma_start(xT_sb[:, 0:4, :], xT_r[:, 0:4, cs])
            nc.sync.dma_start(xT_sb[:, 4:8, :], xT_r[:, 4:8, cs])
            return xT_sb

        prev = None
        xT_cur = load_xT(0)
        for c in range(NCHUNK):
            xT_sb = xT_cur
            if c + 1 < NCHUNK:
                xT_cur = load_xT(c + 1)
            qT_sb = qT_pool.tile([128, ET, CHUNK], F32R, tag="qT")
            for et in range(ET):
                ps = ps_q.tile([128, CHUNK], F32, tag="psq")
                for t in range(ET):
                    nc.tensor.matmul(
                        ps[:],
                        wq_sb[:, t, et * 128 : (et + 1) * 128],
                        xT_sb[:, t, :],
                        start=(t == 0),
                        stop=(t == ET - 1),
                    )
                nc.vector.tensor_scalar(
                    qT_sb[:, et, :],
                    ps[:],
                    bq_sb[:, et : et + 1],
                    None,
                    mybir.AluOpType.add,
                )

            # Attention for chunk c, interleaved with chunk c-1's output
            # projection: the final-matmul groups have no dependency on this
            # chunk's exps, so they keep the PE busy (and the HAM clock-gate
            # warm) while the ScalarE exp of each head pair is in flight.
            #
            # fp32r matmuls must write PSUM at base partition 0, so each head
            # gets its own [64, CHUNK] attnV tile; the even head of a pair is
            # evicted to oT[0:64] by the DVE, the odd head by a
            # partition-shifting SBUF<-SBUF DMA to oT[64:128]. The softmax
            # division happens afterwards, in place on oT.
            pden = ps_den.tile([H, CHUNK], F32, tag="psden")
            oT_sb = oT_pool.tile([128, ET, CHUNK], F32R, tag="oT")
            for et in range(ET):
                hA, hB = 2 * et, 2 * et + 1
                psa = ps_s.tile([SK, CHUNK], F32, tag="pss")
                psb = ps_s.tile([SK, CHUNK], F32, tag="pss")
                # Adjacent score matmuls target PE row groups 0/64 and can
                # overlap in the array.
                nc.tensor.matmul(
                    psa[:], kT_sb[0:64, et, 0:SK], qT_sb[0:64, et, :],
                    start=True, stop=True,
                )
                nc.tensor.matmul(
                    psb[:], kT_sb[64:128, et, 0:SK], qT_sb[64:128, et, :],
                    start=True, stop=True,
                )
                exa = exps_pool.tile([SK, CHUNK], F32R, tag="exps")
                exb = exps_pool.tile([SK, CHUNK], F32R, tag="exps")
                nc.scalar.activation(exa[:], psa[:], mybir.ActivationFunctionType.Exp)
                nc.scalar.activation(exb[:], psb[:], mybir.ActivationFunctionType.Exp)
                for h, ex in ((hA, exa), (hB, exb)):
                    nc.tensor.matmul(
                        pden[:],
                        oh_sb[:, h * H : (h + 1) * H],
                        ex[:],
                        start=(h == 0),
                        stop=(h == H - 1),
                    )
                    pav = ps_av.tile([64, CHUNK], F32, tag="psav")
                    nc.tensor.matmul(
                        pav[:],
                        v_sb[:, h * 64 : (h + 1) * 64],
                        ex[:],
                        start=True,
                        stop=True,
                    )
                    if h == hA:
                        nc.vector.tensor_copy(oT_sb[0:64, et, :], pav[:])
                    else:
                        tmpb = tmpb_pool.tile([64, CHUNK], F32R, tag="tmpb")
                        nc.vector.tensor_copy(tmpb[:], pav[:])
                        nc.sync.dma_start(oT_sb[64:128, et, :], tmpb[:])
                if prev is not None:
                    emit_final_group(prev[0], prev[1], et)
            recip = recip_pool.tile([H, CHUNK], F32R, tag="recip")
            with nc.allow_low_precision(reason="fp32r feeds select-matmul"):
                nc.vector.reciprocal(recip[:], pden[:])
            # Broadcast recip rows (2*et, 2*et+1) across the pair's 128
            # partitions with a one-hot select matmul, then divide in place.
            for et in range(ET):
                rb = ps_s.tile([128, CHUNK], F32, tag="pss")
                nc.tensor.matmul(
                    rb[:],
                    sel_sb[:, et * 128 : (et + 1) * 128],
                    recip[:],
                    start=True,
                    stop=True,
                )
                nc.vector.tensor_tensor(
                    oT_sb[0:64, et, :],
                    oT_sb[0:64, et, :],
                    rb[0:64, :],
                    mybir.AluOpType.mult,
                )
                nc.vector.tensor_tensor(
                    oT_sb[64:128, et, :],
                    oT_sb[64:128, et, :],
                    rb[64:128, :],
                    mybir.AluOpType.mult,
                )
            prev = (c, oT_sb)
        for i in range(8):
            emit_final_group(prev[0], prev[1], i)

    nc.compile()
    return nc


def _get_program():
    global _PROGRAM
    if _PROGRAM is None:
        _PROGRAM = _build_program()
    return _PROGRAM


def kernel(x, y, wq, bq, wk, bk, wv, bv, wo, bo):
    x = np.asarray(x, dtype=np.float32)
    y = np.asarray(y, dtype=np.float32)
    wq = np.asarray(wq, dtype=np.float32)
    bq = np.asarray(bq, dtype=np.float32)
    wk = np.asarray(wk, dtype=np.float32)
    bk = np.asarray(bk, dtype=np.float32)
    wv = np.asarray(wv, dtype=np.float32)
    bv = np.asarray(bv, dtype=np.float32)
    wo = np.asarray(wo, dtype=np.float32)
    bo = np.asarray(bo, dtype=np.float32)

    scale = np.float32(1.0 / np.sqrt(D))
    wq_s = _round_f32r(wq * scale)
    bq_s = (bq * scale).astype(np.float32)

    wk_aug = np.zeros((CA, E), dtype=np.float32)
    wk_aug[:C] = wk
    wk_aug[C] = bk
    wk_aug = _round_f32r(wk_aug)

    wv_aug = np.zeros((CA, H * 64), dtype=np.float32)
    wv_aug[:C] = wv
    wv_aug[C] = bv
    wv_aug = _round_f32r(wv_aug)

    wo_r = _round_f32r(wo)

    onehot = np.zeros((SK, H, H), dtype=np.float32)
    for h in range(H):
        onehot[:, h, h] = 1.0
    onehot = onehot.reshape(SK, H * H)

    sel = np.zeros((H, ET, 128), dtype=np.float32)
    for et in range(ET):
        sel[2 * et, et, 0:64] = 1.0
        sel[2 * et + 1, et, 64:128] = 1.0
    sel = sel.reshape(H, ET * 128)

    nc = _get_program()
    in_maps = []
    for b in range(N_CORES):
        xT = _round_f32r(x[b].T)
        yT = np.zeros((CA, SKP), dtype=np.float32)
        yT[:C, :SK] = y[b].T
        yT[C, :SK] = 1.0
        yT = _round_f32r(yT)
        in_maps.append(
            {
                "xT": xT,
                "yT": yT,
                "wq": wq_s,
                "bq": bq_s,
                "wk": wk_aug,
                "wv": wv_aug,
                "wo": wo_r,
                "bo": bo,
                "oh": onehot,
                "sel": sel,
            }
        )

    trace = bool(int(os.environ.get("KERNEL_TRACE", "0")))
    kwargs = {}
    if trace:
        kwargs = {"trace": True, "tmpdir": os.environ.get("KERNEL_TRACE_DIR")}
    res = run_bass_kernel_spmd(nc, in_maps, list(range(N_CORES)), **kwargs)
    if trace:
        kernel.last_exec_time_ns = res.exec_time_ns
        kernel.last_results = res
    out = np.stack([res.results[b]["out"] for b in range(N_CORES)])
    return np.ascontiguousarray(out)
